# revision 55
# baseline (speedup 1.0000x reference)
"""AbsPosAttention Trainium2 kernel, 8-way sharded (2 batch x 4 head-groups).

Reference (per batch b):
  q = split_heads(x @ Wq) * scale               [H, N, dk]
  k = split_heads(x @ Wk)                       [H, N, dk]
  v = split_heads(x @ Wv)                       [H, N, dv]
  qb = q + pos_embed + rel_content_bias
  out = softmax(qb @ k^T) @ v                   per head
  y = concat_heads(out) @ Wo + bo

Sharding: core c = 4*b + g computes batch b, heads {2g, 2g+1}; host sums the
4 group partials per batch, transposes (device emits y^T) and adds bo.

All bulk inputs ship bf16 (x/Wq/Wk/Wv host-cast; Wq/Wk/Wv host-rearranged
into SBUF layout so each is one big-line DMA); posb (pos+rcb, the
logit-dominant term) stays f32. qT/kT stay f32r on-chip so the logits see
only the input rounding. Measured rel err ~8.5e-3 vs the 2e-2 gate.

Per-core structure (matmuls contract over the SBUF partition dim,
out = lhsT.T @ rhs):
  - xt arrives as 48 per-(chunk, i-block) pieces, i-block-major, on the
    sync+gpsimd queues only (bulk DMA on the scalar queue would convoy
    the exp pipeline behind ring backpressure). EXP/LN activation tables
    pre-warmed during the DMA window.
  - Q(ib)/K(ib) projections are [128,512] psum tiles on the lg rotation,
    interleaved one i-block ahead into the merged loop below; evictions
    add posb (q) / copy (k) into f32r qT/kT.
  - Merged A2/B-ib0 loop: per j-tile: V projection (vps on lg rotation)
    -> both heads' logits^T in one [128,1024] 2-bank psum (two K=64
    matmuls, row groups 0/64) -> Exp -> pt bf16; AV accumulation chases
    one j-tile behind. V evicts to v_all bf16 with a trailing ones column
    per head ([v | 1], 194-stride) for the softmax denominator.
  - B i-blocks 1..3: lg/exp pipelined 2 j-tiles ahead of AV (av1: v rows
    0..127, av2: v rows 128..191 + denominator row; psum p0..p3); issued
    in 2-j-tile steps [lg, lg, 8xAV] to halve the bf16<->f32 PE
    weight-set transitions (~100ns pipeline drain each).
  - Boundary per ib: raw-evict av psum on vector (av-consumption order);
    denominator reciprocal = exp(-ln Z) on scalar straight off the psum
    row; 1/Z broadcast to [128,512] via DRAM round-trip DMA on gpsimd
    (SBUF repeat-read DMA costs ~12us; DRAM-source broadcast ~1us) --
    last boundary instead matmul-broadcasts through the idle lg rotation
    to keep phase C's ib3 chain short. Normalize tts run on gpsimd
    mid-phase (keeps vector clear for the next boundary's evicts), on
    vector for the last; partition-up-shift pieces stage via stg + DMA
    (DVE shifts down only).
  - C: ib-major so columns 0..2 never wait on the last boundary; per
    4-e group an 8-deep psum rotation (lg pairs / p quads); casts split
    scalar/vector, y DMAs split sync/gpsimd; y ships bf16.
"""

import numpy as np

HEADS, DIM_KEY, DIM_VALUE, DIM, N, B = 8, 64, 192, 1536, 2048, 2
SCALE = DIM_KEY**-0.5
NCORES, GROUPS, HPC = 8, 4, 2
NCH = DIM // 128  # 12 contraction chunks for the projections
NIB = N // 512  # 4 i-blocks
NJT = N // 128  # 16 j-tiles
DVC = HPC * DIM_VALUE  # 384
VH = DIM_VALUE + 1  # 193: [v | 1] per head
VHP = VH + 1  # 194: padded per-head stride in v_all
VJ = 2 * VHP  # 388: per-j-tile stride
NEC = DIM // 128  # 12 e-chunks in phase C

_cached = {}


def _install_patches():
    """Work around this walrus build's 1-sync-wait-per-instruction limit."""
    import concourse.tile as _tile
    from concourse import mybir

    def _drain_and_barrier(self, tick_clock, wait_clock):
        nc = self.nc
        probe = nc.sync.nop(nofuse=True, hint="tail_drain_waits")
        wait_clock.add_sem_waits(
            probe.ins, _tile.ScopedClock({None: tick_clock.global_clock})
        )
        si = probe.ins.sync_info
        waits = list(si.on_wait) if si and si.on_wait else []
        if len(waits) > 1:
            probe.ins.sync_info.on_wait = waits[:1]
            for w in waits[1:]:
                extra = nc.sync.nop(nofuse=True, hint="tail_drain_waits")
                esi = extra.ins.sync_info
                if esi is None:
                    extra.ins.sync_info = mybir.SyncInfo(on_wait=[w], on_update=[])
                else:
                    esi.on_wait = [w]
        nc.sync.drain()
        nc.all_engine_barrier()
        assert self.sems is not None
        popped = nc._tile_sem_poison_stack.pop()
        assert popped is self._sem_poison
        nc.clear_and_free_semaphores(list(self.sems.allocated().values()))
        nc.all_engine_barrier()

    _tile.TileContext._drain_and_barrier = _drain_and_barrier


def _split_sync_waits(nc, max_waits=1):
    from concourse import mybir

    for f in nc.m.functions:
        for bb in f.blocks:
            insts = list(bb.instructions)
            out = []
            changed = False
            for inst in insts:
                si = getattr(inst, "sync_info", None)
                if si is not None and si.on_wait and len(si.on_wait) > max_waits:
                    waits = list(si.on_wait)
                    extra, keep = waits[:-max_waits], waits[-max_waits:]
                    si.on_wait = keep
                    for i in range(0, len(extra), max_waits):
                        out.append(
                            mybir.InstNoOp(
                                name=nc.get_next_instruction_name(),
                                engine=inst.engine,
                                ins=[],
                                outs=[],
                                sync_info=mybir.SyncInfo(
                                    on_wait=extra[i : i + max_waits], on_update=[]
                                ),
                                bass_nofuse=True,
                            )
                        )
                    changed = True
                out.append(inst)
            if changed:
                bb.instructions[:] = out


def _build(split_waits=True):
    from contextlib import ExitStack

    import concourse.bass as bass
    import concourse.tile as tile
    from concourse import mybir
    from concourse.bass import ts

    _install_patches()

    f32 = mybir.dt.float32
    f32r = mybir.dt.float32r
    bf16 = mybir.dt.bfloat16
    EXP = mybir.ActivationFunctionType.Exp
    CPY = mybir.ActivationFunctionType.Copy
    LN = mybir.ActivationFunctionType.Ln
    MULT = mybir.AluOpType.mult

    nc = bass.Bass()
    zscr = nc.dram_tensor("zscr", [2 * NIB, 512], f32, kind="Internal")
    xt = nc.dram_tensor("xt", [DIM, N], bf16, kind="ExternalInput")
    # wq/wk/wv pre-rearranged on the host into SBUF layout so each loads
    # with ONE big-line DMA (3KB/9KB partition lines vs 256B fragments).
    wq = nc.dram_tensor("wq", [128, NCH * 128], bf16, kind="ExternalInput")
    wk = nc.dram_tensor("wk", [128, NCH * 128], bf16, kind="ExternalInput")
    wv = nc.dram_tensor("wv", [128, NCH * DVC], bf16, kind="ExternalInput")
    posb = nc.dram_tensor("posb", [128, N], f32, kind="ExternalInput")
    wo = nc.dram_tensor("wo", [DVC, DIM], bf16, kind="ExternalInput")
    y = nc.dram_tensor("y", [DIM, N], bf16, kind="ExternalOutput")

    from concourse import library_config

    with tile.TileContext(nc) as tc:
        with ExitStack() as ctx:
            sb = ctx.enter_context(tc.tile_pool(name="sb", bufs=1))
            ps = ctx.enter_context(tc.tile_pool(name="ps", bufs=1, space="PSUM"))

            # ---- persistent SBUF + input DMA (multi-queue) -------------
            # xt arrives as 48 per-(chunk, i-block) pieces in i-block-major
            # order across 3 queues; Tile's sub-tile write tracking lets
            # each QK matmul wait only on its own piece, so the pipeline
            # spins up as soon as i-block 0's 1.5MB lands.
            xt_sb = [
                sb.tile([128, N], bf16, name=f"xts{c}", tag=f"xts{c}")
                for c in range(NCH)
            ]
            wq_sb = sb.tile([128, NCH * 128], bf16, tag="wq")
            wk_sb = sb.tile([128, NCH * 128], bf16, tag="wk")
            wv_sb = sb.tile([128, NCH * DVC], bf16, tag="wv")
            posb_sb = sb.tile([128, N], f32, tag="posb")
            wo_sb = sb.tile([128, 3 * DIM], bf16, tag="wo")
            # All input DMAs ride sync+gpsimd in need-time order (the DMA
            # engines fair-share bandwidth across queues, so any transfer
            # issued early steals bandwidth from the critical first
            # i-block). The scalar queue carries no bulk DMA -- it would
            # convoy the exp pipeline behind ring backpressure.
            def xt_pieces(ib):
                for c in range(NCH):
                    QS[c % 2].dma_start(
                        xt_sb[c][:, ts(ib, 512)], xt[ts(c, 128), ts(ib, 512)]
                    )

            QS = [nc.sync, nc.gpsimd]
            WVH = 6 * DVC
            nc.sync.dma_start(wq_sb[:], wq[:, :])
            nc.sync.dma_start(posb_sb[:, 0:512], posb[:, 0:512])
            xt_pieces(0)
            nc.gpsimd.dma_start(wk_sb[:], wk[:, :])
            nc.gpsimd.dma_start(wv_sb[:, WVH:], wv[:, WVH:])
            nc.sync.dma_start(wv_sb[:, 0:WVH], wv[:, 0:WVH])
            xt_pieces(1)
            nc.sync.dma_start(posb_sb[:, 512:1024], posb[:, 512:1024])
            xt_pieces(2)
            nc.sync.dma_start(posb_sb[:, 1024:2048], posb[:, 1024:2048])
            xt_pieces(3)
            for k in range(3):
                nc.sync.dma_start(wo_sb[:, ts(k, DIM)], wo[ts(k, 128), :])

            qT = sb.tile([128, N], f32r, tag="qT")
            kT = sb.tile([128, N], f32r, tag="kT")
            v_all = sb.tile([128, NJT * VJ], bf16, tag="v_all")
            o_sb = sb.tile([128, 3 * N], bf16, tag="o_sb")

            ones_view = v_all[:].rearrange("p (j h c) -> p j h c", j=NJT, h=HPC)
            nc.vector.memset(ones_view[:, :, :, 192:193], 1.0)
            ones_t = sb.tile([128, 128], f32, tag="ones_t")
            nc.vector.memset(ones_t[:], 1.0)
            # pre-warm the scalar engine's EXP/LN tables during the DMA
            # window (a lazy ACT_TABLE_LOAD costs 1.5us on the first
            # activation, which otherwise lands on the critical path).
            warm = sb.tile([1, 16], f32, tag="warm")
            nc.scalar.activation(warm[:], ones_t[0:1, 0:16], EXP)
            nc.scalar.activation(warm[:], ones_t[0:1, 0:16], LN)

            # ---- Phase A1: per-i-block Q/K on the lg psum rotation -----
            # Q(ib)/K(ib) are [128,512] tiles cycling the two lg slots;
            # interleaved into the merged loop below so the rotation order
            # matches data-arrival order.
            def qk_proj(ib):
                qp = ps.tile([128, 512], f32, name=f"qps{ib}", tag="lg", bufs=2)
                for c in range(NCH):
                    nc.tensor.matmul(
                        qp[:],
                        wq_sb[:, ts(c, 128)],
                        xt_sb[c][:, ts(ib, 512)],
                        start=(c == 0),
                        stop=(c == NCH - 1),
                    )
                nc.vector.tensor_add(
                    qT[:, ts(ib, 512)], qp[:], posb_sb[:, ts(ib, 512)]
                )
                kp = ps.tile([128, 512], f32, name=f"kps{ib}", tag="lg", bufs=2)
                for c in range(NCH):
                    nc.tensor.matmul(
                        kp[:],
                        wk_sb[:, ts(c, 128)],
                        xt_sb[c][:, ts(ib, 512)],
                        start=(c == 0),
                        stop=(c == NCH - 1),
                    )
                nc.vector.tensor_copy(kT[:, ts(ib, 512)], kp[:])

            def v_proj(j):
                vps = ps.tile([128, DVC], f32, name="vps", tag="lg", bufs=2)
                for c in range(NCH):
                    nc.tensor.matmul(
                        vps[:],
                        xt_sb[c][:, ts(j, 128)],
                        wv_sb[:, ts(c, DVC)],
                        start=(c == 0),
                        stop=(c == NCH - 1),
                    )
                for h in range(HPC):
                    nc.vector.tensor_copy(
                        v_all[:, j * VJ + h * VHP : j * VJ + h * VHP + 192],
                        vps[:, ts(h, 192)],
                    )

            def lg_exp(ib, j):
                lgt = ps.tile([128, 1024], f32, name="lg", tag="lg", bufs=2)
                for h in range(HPC):
                    nc.tensor.matmul(
                        lgt[:, ts(h, 512)],
                        kT[ts(h, 64), ts(j, 128)],
                        qT[ts(h, 64), ts(ib, 512)],
                        start=True,
                        stop=True,
                    )
                pt = sb.tile([128, 1024], bf16, name="pt", tag="pt", bufs=6)
                nc.scalar.activation(pt[:], lgt[:], EXP)
                return pt

            def av_mm(av1, av2, pt, j, start, stop):
                for h in range(HPC):
                    nc.tensor.matmul(
                        av1[h][:],
                        v_all[:, j * VJ + h * VHP : j * VJ + h * VHP + 128],
                        pt[:, ts(h, 512)],
                        start=start,
                        stop=stop,
                    )
                    nc.tensor.matmul(
                        av2[h][:],
                        v_all[:, j * VJ + h * VHP + 128 : j * VJ + h * VHP + VH],
                        pt[:, ts(h, 512)],
                        start=start,
                        stop=stop,
                    )

            rcb = [
                sb.tile([128, 512], f32, name=f"rcb{h}", tag=f"rcb{h}", bufs=2)
                for h in range(HPC)
            ]

            def boundary(ib, av1, av2, last=False):
                # Raw-evict av psum fast (unblocks av for ib+1), then
                # normalize out-of-band split across vector/gpsimd while
                # the next i-block runs.
                raw1 = [
                    sb.tile([128, 512], f32, name=f"raw1_{h}", tag=f"raw1_{h}", bufs=2)
                    for h in range(HPC)
                ]
                raw2 = [
                    sb.tile([128, 512], f32, name=f"raw2_{h}", tag=f"raw2_{h}", bufs=2)
                    for h in range(HPC)
                ]
                # evict in av-consumption order so next i-block's first
                # AV matmuls unblock as early as possible
                for h in range(HPC):
                    nc.vector.tensor_copy(raw1[h][:], av1[h][:])
                    nc.vector.tensor_copy(raw2[h][0:64, :], av2[h][0:64, :])
                for h in range(HPC):
                    # Denominator reciprocal as exp(-ln Z) on the scalar
                    # engine straight off the PSUM row; broadcast via a
                    # free-dim-repeat DMA on gpsimd. Keeps the boundary off
                    # the lg-psum rotation and the slow single-lane DVE
                    # reciprocal off the DVE queue.
                    rln = sb.tile(
                        [1, 512], f32, name=f"rln{h}", tag=f"rln{h}", bufs=2
                    )
                    rrow = sb.tile(
                        [1, 512], f32, name=f"rrow{h}", tag=f"rrow{h}", bufs=2
                    )
                    nc.scalar.activation(rln[:], av2[h][64:65, :], LN)
                    nc.scalar.activation(rrow[:], rln[:], EXP, scale=-1.0)
                    if last:
                        # lg rotation is idle after the final i-block:
                        # matmul-broadcast is fast (keeps phase C's ib3
                        # column chain short).
                        rps = ps.tile(
                            [128, 512], f32, name=f"rps{h}", tag="lg", bufs=2
                        )
                        nc.tensor.matmul(
                            rps[:], ones_t[0:1, :], rrow[:], start=True, stop=True
                        )
                        nc.vector.tensor_copy(rcb[h][:], rps[:])
                    else:
                        # mid-phase: broadcast via DRAM round-trip (a
                        # repeat-read straight from SBUF costs ~12us; the
                        # DRAM-source broadcast read is ~10x faster).
                        idx = ib * HPC + h
                        nc.gpsimd.dma_start(zscr[idx : idx + 1, :], rrow[:])
                        zt = zscr[idx : idx + 1, :]
                        bc = bass.AP(
                            tensor=zt.tensor,
                            offset=zt.offset,
                            ap=[[0, 128]] + list(zt.ap[1:]),
                        )
                        nc.gpsimd.dma_start(rcb[h][:], bc)
                # Normalize into o_sb. DVE can shift partitions DOWN but
                # not up, so the two pieces landing at partitions 64:128
                # stage through stg + DMA. Mid-phase the tts run on gpsimd
                # (keeps the vector queue clear for the NEXT boundary's
                # raw evicts); the last boundary uses the idle vector.
                te = nc.vector if last else nc.gpsimd
                o0 = ib * 512
                te.tensor_tensor(
                    o_sb[:, o0 : o0 + 512], raw1[0][:], rcb[0][:], MULT
                )
                te.tensor_tensor(
                    o_sb[0:64, N + o0 : N + o0 + 512],
                    raw2[0][0:64, :], rcb[0][0:64, :], MULT,
                )
                stg = sb.tile([64, 1024], bf16, name="stg", tag="stg", bufs=2)
                te.tensor_tensor(
                    stg[0:64, 0:512], raw1[1][0:64, :], rcb[1][0:64, :], MULT
                )
                (nc.gpsimd if last else te).tensor_tensor(
                    o_sb[0:64, 2 * N + o0 : 2 * N + o0 + 512],
                    raw1[1][64:128, :], rcb[1][64:128, :], MULT,
                )
                (nc.gpsimd if last else te).tensor_tensor(
                    stg[0:64, 512:1024], raw2[1][0:64, :], rcb[1][0:64, :], MULT
                )
                nc.sync.dma_start(
                    o_sb[64:128, N + o0 : N + o0 + 512], stg[0:64, 0:512]
                )
                nc.sync.dma_start(
                    o_sb[64:128, 2 * N + o0 : 2 * N + o0 + 512],
                    stg[0:64, 512:1024],
                )

            # ---- Phase A2 merged with attention i-block 0 --------------
            # QK(ib) feeds in one i-block ahead of the lg j-tiles that
            # need it; V production chases the AV consumption j-tile by
            # j-tile.
            av1 = [
                ps.tile([128, 512], f32, name=f"av1_{h}", tag=f"p{h}")
                for h in range(HPC)
            ]
            av2 = [
                ps.tile([65, 512], f32, name=f"av2_{h}", tag=f"p{2 + h}")
                for h in range(HPC)
            ]
            qk_proj(0)
            pt_q = []
            for j in range(NJT):
                if 1 <= j <= 3:
                    qk_proj(j)
                v_proj(j)
                pt_q.append(lg_exp(0, j))
                if j >= 1:
                    av_mm(av1, av2, pt_q.pop(0), j - 1, j - 1 == 0, False)
            av_mm(av1, av2, pt_q.pop(0), NJT - 1, False, True)
            boundary(0, av1, av2)

            # ---- Phase B: attention i-blocks 1..3 ----------------------
            flat = [(ib, j) for ib in range(1, NIB) for j in range(NJT)]

            def lg_exp_flat(idx):
                if idx < len(flat):
                    return lg_exp(*flat[idx])
                return None

            pt_q = [lg_exp_flat(0), lg_exp_flat(1)]
            for ib in range(1, NIB):
                av1 = [
                    ps.tile([128, 512], f32, name=f"av1_{h}", tag=f"p{h}")
                    for h in range(HPC)
                ]
                av2 = [
                    ps.tile([65, 512], f32, name=f"av2_{h}", tag=f"p{2 + h}")
                    for h in range(HPC)
                ]
                # 2-j-tile steps: [lg pair, lg pair, 8 AVs] halves the
                # bf16<->f32 PE weight-set transitions (~100ns drain each).
                for j2 in range(0, NJT, 2):
                    base = (ib - 1) * NJT + j2
                    pt_q.append(lg_exp_flat(base + 2))
                    pt_a = pt_q.pop(0)
                    pt_q.append(lg_exp_flat(base + 3))
                    pt_b = pt_q.pop(0)
                    av_mm(av1, av2, pt_a, j2, j2 == 0, False)
                    av_mm(av1, av2, pt_b, j2 + 1, False, j2 + 1 == NJT - 1)
                boundary(ib, av1, av2, last=(ib == NIB - 1))

            # ---- Phase C: output projection (y^T = wo^T @ O^T) ---------
            # ib-major: columns for i-blocks 0..2 have long-ready o_sb, so
            # they never wait on the final boundary's normalize chain.
            # 8-deep psum rotation: 4-e groups alternate lg-pairs / p-quads.
            grp = 0
            for ibc in range(NIB):
                for e0 in range(0, NEC, 4):
                    if grp % 2 == 0:
                        yps = [
                            ps.tile([128, 1024], f32, name=f"yg{p}", tag="lg", bufs=2)
                            for p in range(2)
                        ]
                        youts = [
                            yps[p][:, ts(i, 512)] for p in range(2) for i in range(2)
                        ]
                    else:
                        yp4 = [
                            ps.tile([128, 512], f32, name=f"yp{p}", tag=f"p{p}")
                            for p in range(4)
                        ]
                        youts = [t[:] for t in yp4]
                    grp += 1
                    for el in range(4):
                        e = e0 + el
                        for k in range(3):
                            nc.tensor.matmul(
                                youts[el],
                                wo_sb[:, k * DIM + e * 128 : k * DIM + e * 128 + 128],
                                o_sb[:, k * N + ibc * 512 : k * N + ibc * 512 + 512],
                                start=(k == 0),
                                stop=(k == 2),
                            )
                    for el in range(4):
                        e = e0 + el
                        yo = sb.tile([128, 512], bf16, name="yo", tag="yo", bufs=8)
                        if el % 2 == 0:
                            nc.scalar.activation(yo[:], youts[el], CPY)
                        else:
                            nc.vector.tensor_copy(yo[:], youts[el])
                        # y DMAs on the hardware-DGE queues only: the
                        # gpsimd software-DGE queue's end-of-kernel drain
                        # walks every issued DMA (~100ns each). The last
                        # column splits onto scalar (free by then) so the
                        # sync issue backlog (~600ns/DMA) doesn't trail
                        # past the final matmul.
                        if ibc == NIB - 1 and el % 2 == 1:
                            yq = nc.scalar
                        else:
                            yq = nc.sync
                        yq.dma_start(y[ts(e, 128), ts(ibc, 512)], yo[:])

    if split_waits:
        _split_sync_waits(nc)
    return nc


def _shard_inputs(x, Wq, Wk, Wv, Wo, pos_embed, rel_content_bias):
    import ml_dtypes

    bfloat16 = ml_dtypes.bfloat16
    in_maps = []
    xts = [np.ascontiguousarray(x[b].T).astype(bfloat16) for b in range(B)]
    for c in range(NCORES):
        b, g = divmod(c, GROUPS)
        h0 = g * HPC
        wq_l = np.ascontiguousarray(Wq[:, h0 * DIM_KEY : (h0 + HPC) * DIM_KEY]) * SCALE
        wk_l = np.ascontiguousarray(Wk[:, h0 * DIM_KEY : (h0 + HPC) * DIM_KEY])
        wv_l = np.ascontiguousarray(Wv[:, h0 * DIM_VALUE : (h0 + HPC) * DIM_VALUE])
        pp = (
            pos_embed[h0 : h0 + HPC] + rel_content_bias[0, h0 : h0 + HPC]
        )  # [2, N, dk]
        posb = np.ascontiguousarray(pp.transpose(0, 2, 1)).reshape(128, N)
        wo_l = np.ascontiguousarray(
            Wo[h0 * DIM_VALUE : (h0 + HPC) * DIM_VALUE]
        ).astype(bfloat16)
        # rearrange to SBUF layout: [128, nch*free], block c = rows 128c..
        wq_r = wq_l.reshape(NCH, 128, 128).transpose(1, 0, 2).reshape(128, -1)
        wk_r = wk_l.reshape(NCH, 128, 128).transpose(1, 0, 2).reshape(128, -1)
        wv_r = wv_l.reshape(NCH, 128, DVC).transpose(1, 0, 2).reshape(128, -1)
        in_maps.append(
            {
                "xt": xts[b],
                "wq": np.ascontiguousarray(wq_r).astype(bfloat16),
                "wk": np.ascontiguousarray(wk_r).astype(bfloat16),
                "wv": np.ascontiguousarray(wv_r).astype(bfloat16),
                "posb": posb.astype(np.float32),
                "wo": wo_l,
            }
        )
    return in_maps


def kernel(x, Wq, Wk, Wv, Wo, bo, pos_embed, rel_content_bias, _trace=False):
    from concourse.bass_utils import run_bass_kernel_spmd

    x = np.asarray(x, np.float32)
    Wq = np.asarray(Wq, np.float32)
    Wk = np.asarray(Wk, np.float32)
    Wv = np.asarray(Wv, np.float32)
    Wo = np.asarray(Wo, np.float32)
    bo = np.asarray(bo, np.float32)
    pos_embed = np.asarray(pos_embed, np.float32)
    rel_content_bias = np.asarray(rel_content_bias, np.float32)

    if "nc" not in _cached:
        _cached["nc"] = _build()
    nc = _cached["nc"]

    in_maps = _shard_inputs(x, Wq, Wk, Wv, Wo, pos_embed, rel_content_bias)
    res = run_bass_kernel_spmd(
        nc, in_maps, core_ids=list(range(NCORES)), trace=_trace
    )
    _cached["last_result"] = res

    out = np.zeros((B, N, DIM), np.float32)
    for b in range(B):
        acc = res.results[b * GROUPS]["y"].astype(np.float32)
        for g in range(1, GROUPS):
            acc = acc + res.results[b * GROUPS + g]["y"].astype(np.float32)
        out[b] = acc.T + bo[None, :]
    return out

